# revision 22
# baseline (speedup 1.0000x reference)
"""Trainium2 Bass kernel for DeformableConv2d (B,H,W,C=8,64,64,128; F=128; 3x3).

Strategy (data-parallel over batch, one batch element per NeuronCore):
  - Host: reshape inputs, precompute the (data-independent) base-grid constant
    and a channel-major weight layout. No data-dependent work on host.
  - Device, per core:
      * build x_pair in scratch DRAM: row q -> [x[q], x[q+64]] (zero padded),
        so one 512-element contiguous read at offset q*256 fetches the whole
        2x2 bilinear patch for integer corner q = y0*64 + x0.
      * index math on DVE: coords = grid + offset, clip, frac via mod,
        q = y0*64 + x0 (int32), 4 bilinear corner weights.
      * per pixel-tile (128 px): one indirect DMA gathers all 9 kernel
        points' patches [128, 9, 512]; DVE combines the 4 corners with
        per-partition scalar weights; PE transposes deform tiles into PSUM
        (channel-major); PE matmuls accumulate over the 9 kernel points into
        out^T; PE transposes back and results stream to DRAM.
"""

import os
from contextlib import ExitStack

import numpy as np

import concourse.bass as bass
import concourse.mybir as mybir
import concourse.tile as tile
from concourse import bacc
from concourse._compat import with_exitstack
from concourse.bass_utils import run_bass_kernel_spmd
from concourse.masks import make_identity

KH, KW, KN = 3, 3, 9
H = W_IMG = 64
C = 128
F = 128
P = 128
NPIX = H * W_IMG            # 4096 pixels per core
NT = NPIX // P              # 32 pixel tiles
NG = NT // 4                # 8 groups of 512 pixels
XROWS = NPIX + 1            # x rows incl. one zero pad row (host-padded)

f32 = mybir.dt.float32
i32 = mybir.dt.int32
ALU = mybir.AluOpType
ACT = mybir.ActivationFunctionType


def _grid_const():
    """reference._grid_offset in numpy, flattened to [4096, 18] then wrapped
    to the [128 partitions, 32*18] on-chip layout."""
    init = np.stack(np.meshgrid(np.arange(KH), np.arange(KW), indexing="ij"))
    init = init.reshape(-1, 2).astype(np.float32)
    ph, pw = (KH - 1) // 2, (KW - 1) // 2
    g = np.stack(
        np.meshgrid(np.arange(-ph, H - ph), np.arange(-pw, W_IMG - pw), indexing="ij"),
        axis=-1,
    ).astype(np.float32)
    full = (g[:, :, None, :] + init[None, None]).reshape(NPIX, 2 * KN)
    return np.ascontiguousarray(
        full.reshape(NT, P, 2 * KN).transpose(1, 0, 2).reshape(P, NT * 2 * KN)
    )


@with_exitstack
def _body(ctx: ExitStack, tc: "tile.TileContext", t_x, t_off, t_grid, t_w, t_b,
          t_out, debug=False):
    nc = tc.nc
    x_ap = t_x.ap()
    off_ap = t_off.ap()
    grid_ap = t_grid.ap()
    w_ap = t_w.ap()
    b_ap = t_b.ap()
    out_ap = t_out.ap()

    const = ctx.enter_context(tc.tile_pool(name="const", bufs=1))
    idxp = ctx.enter_context(tc.tile_pool(name="idx", bufs=1))
    gpool = ctx.enter_context(tc.tile_pool(name="gath", bufs=8))
    dpool = ctx.enter_context(tc.tile_pool(name="deform", bufs=4))
    dTpool = ctx.enter_context(tc.tile_pool(name="dT", bufs=3))
    oTpool = ctx.enter_context(tc.tile_pool(name="oT", bufs=2))
    opool = ctx.enter_context(tc.tile_pool(name="o", bufs=4))
    ps_out = ctx.enter_context(tc.tile_pool(name="ps_out", bufs=2, space="PSUM"))
    ps_dT = ctx.enter_context(tc.tile_pool(name="ps_dT", bufs=2, space="PSUM"))
    ps_o = ctx.enter_context(tc.tile_pool(name="ps_o", bufs=2, space="PSUM"))

    # ---- constants ----
    ident = const.tile([P, P], f32)
    make_identity(nc, ident[:])
    w_sb = const.tile([P, KN, F], f32)
    nc.sync.dma_start(w_sb[:], w_ap)  # [C, KN, F], c on partitions
    b_sb = const.tile([P, 1], f32)
    nc.sync.dma_start(b_sb[:], b_ap[:, None])

    # ---- load offsets + grid ----
    offs = idxp.tile([P, NT, 2 * KN], f32)
    nc.sync.dma_start(offs[:], off_ap.rearrange("(t p) k -> p t k", p=P))
    grid = idxp.tile([P, NT, 2 * KN], f32)
    nc.sync.dma_start(grid[:], grid_ap.rearrange("p (t k) -> p t k", k=2 * KN))

    # ---- index math (all tiles at once) ----
    co = idxp.tile([P, NT, 2 * KN], f32)
    nc.vector.tensor_add(co[:], offs[:], grid[:])
    nc.vector.tensor_scalar(co[:], co[:], 0.0, float(H - 1), ALU.max, ALU.min)
    # floor via int round-trip; works for round-to-nearest (HW) and trunc (sim):
    # r = float(int(y)); floor = r - (r > y)
    ci = idxp.tile([P, NT, 2 * KN], i32)
    nc.vector.tensor_copy(ci[:], co[:])
    cf = idxp.tile([P, NT, 2 * KN], f32)
    nc.vector.tensor_copy(cf[:], ci[:])
    gt = idxp.tile([P, NT, 2 * KN], f32)
    nc.vector.tensor_tensor(gt[:], cf[:], co[:], ALU.is_gt)
    c0 = idxp.tile([P, NT, 2 * KN], f32)
    nc.vector.tensor_sub(c0[:], cf[:], gt[:])
    fr = idxp.tile([P, NT, 2 * KN], f32)
    nc.vector.tensor_sub(fr[:], co[:], c0[:])
    un = idxp.tile([P, NT, 2 * KN], f32)
    nc.vector.tensor_scalar(un[:], fr[:], -1.0, 1.0, ALU.mult, ALU.add)

    c0v = c0[:].rearrange("p t (n two) -> p t n two", two=2)
    frv = fr[:].rearrange("p t (n two) -> p t n two", two=2)
    unv = un[:].rearrange("p t (n two) -> p t n two", two=2)

    qf = idxp.tile([P, NT, KN], f32)
    nc.vector.scalar_tensor_tensor(
        qf[:], c0v[:, :, :, 0], 64.0, c0v[:, :, :, 1], ALU.mult, ALU.add
    )
    # hi row-pair start: min(q + 64, NPIX - 1); weight-0 whenever the clamp
    # engages (y0 == 63 implies fy == 0)
    qh = idxp.tile([P, NT, KN], f32)
    nc.vector.tensor_scalar(qh[:], qf[:], 64.0, float(NPIX - 1), ALU.add, ALU.min)
    # combined int index tile, kn-major: qall[p, kn, g, 0:4] = lo idx of the
    # 4 tiles of group g, [4:8] = hi idx — contiguous [128, 8] slices for DMA
    qall = idxp.tile([P, KN, NG, 8], i32)
    qfv = qf[:].rearrange("p (g j) n -> p n g j", j=4)
    qhv = qh[:].rearrange("p (g j) n -> p n g j", j=4)
    nc.vector.tensor_copy(qall[:, :, :, 0:4], qfv)
    nc.vector.tensor_copy(qall[:, :, :, 4:8], qhv)

    # corner weights [00, 10, 01, 11]; rows ~ y (index 0), cols ~ x (index 1)
    w4 = idxp.tile([P, NT, KN, 4], f32)
    nc.vector.tensor_tensor(w4[:, :, :, 0], unv[:, :, :, 0], unv[:, :, :, 1], ALU.mult)
    nc.vector.tensor_tensor(w4[:, :, :, 1], frv[:, :, :, 0], unv[:, :, :, 1], ALU.mult)
    nc.vector.tensor_tensor(w4[:, :, :, 2], unv[:, :, :, 0], frv[:, :, :, 1], ALU.mult)
    nc.vector.tensor_tensor(w4[:, :, :, 3], frv[:, :, :, 0], frv[:, :, :, 1], ALU.mult)

    if debug:
        d_q = nc.dram_tensor("dbg_q", [P, KN * NG * 8], i32, kind="ExternalOutput")
        d_w4 = nc.dram_tensor("dbg_w4", [P, NT * KN * 4], f32, kind="ExternalOutput")
        d_g = nc.dram_tensor("dbg_g", [P, 4 * C], f32, kind="ExternalOutput")
        d_dt = nc.dram_tensor("dbg_dt", [P, 512], f32, kind="ExternalOutput")
        d_ot = nc.dram_tensor("dbg_ot", [P, 512], f32, kind="ExternalOutput")
        nc.sync.dma_start(d_q.ap().rearrange("p (n g j) -> p n g j", n=KN, g=NG), qall[:])
        nc.sync.dma_start(
            d_w4.ap().rearrange("p (t n j) -> p t n j", t=NT, n=KN), w4[:]
        )

    # ---- main loop ----
    for g in range(NG):
        ops = ps_out.tile([P, 512], f32)  # out^T accumulator [f, 512 px]
        for kn in range(KN):
            dps = ps_dT.tile([P, 512], f32)  # deform^T [c, 512 px]
            for t4 in range(4):
                t = g * 4 + t4
                # gather row-pairs: lo = rows (q, q+1) -> corners (00 | 01),
                # hi = rows (q+64, q+65) -> corners (10 | 11).
                # HW indirect DMA needs a single-column [128, 1] offset AP.
                Glo = gpool.tile([P, 2 * C], f32)
                nc.gpsimd.indirect_dma_start(
                    out=Glo[:], out_offset=None, in_=x_ap[:, :],
                    in_offset=bass.IndirectOffsetOnAxis(
                        ap=qall[:, kn, g, t4 : t4 + 1], axis=0),
                )
                Ghi = gpool.tile([P, 2 * C], f32)
                nc.gpsimd.indirect_dma_start(
                    out=Ghi[:], out_offset=None, in_=x_ap[:, :],
                    in_offset=bass.IndirectOffsetOnAxis(
                        ap=qall[:, kn, g, 4 + t4 : 5 + t4], axis=0),
                )
                d = dpool.tile([P, C], f32)
                nc.vector.tensor_scalar_mul(d[:], Glo[:, 0:C], w4[:, t, kn, 0:1])
                for Gt, csel, wsel in ((Glo, C, 2), (Ghi, 0, 1), (Ghi, C, 3)):
                    nc.vector.scalar_tensor_tensor(
                        d[:],
                        Gt[:, csel : csel + C],
                        w4[:, t, kn, wsel : wsel + 1],
                        d[:],
                        ALU.mult,
                        ALU.add,
                    )
                nc.tensor.transpose(dps[:, t4 * P : (t4 + 1) * P], d[:], ident[:])
                if debug and g == 0 and kn == 0 and t4 == 0:
                    nc.sync.dma_start(d_g.ap()[:, 0 : 2 * C], Glo[:])
                    nc.sync.dma_start(d_g.ap()[:, 2 * C : 4 * C], Ghi[:])
            dT = dTpool.tile([P, 512], f32)
            nc.scalar.copy(dT[:], dps[:])
            nc.tensor.matmul(
                ops[:], lhsT=w_sb[:, kn, :], rhs=dT[:],
                start=(kn == 0), stop=(kn == KN - 1),
            )
            if debug and g == 0 and kn == 0:
                nc.sync.dma_start(d_dt.ap(), dT[:])
        oT = oTpool.tile([P, 512], f32)
        nc.scalar.activation(oT[:], ops[:], ACT.Identity, bias=b_sb[:, 0:1], scale=1.0)
        if debug and g == 0:
            nc.sync.dma_start(d_ot.ap(), oT[:])
        for t4 in range(4):
            o_ps = ps_o.tile([P, P], f32)
            nc.tensor.transpose(o_ps[:], oT[:, t4 * P : (t4 + 1) * P], ident[:])
            o_sb = opool.tile([P, P], f32)
            nc.scalar.copy(o_sb[:], o_ps[:])
            pix0 = (g * 4 + t4) * P
            nc.sync.dma_start(out_ap[pix0 : pix0 + P, :], o_sb[:])


def build_nc(debug=False):
    nc = bacc.Bacc(
        "TRN2",
        target_bir_lowering=False,
        debug=False,
        enable_asserts=False,
        num_devices=8,
    )
    t_x = nc.dram_tensor("x", [XROWS, C], f32, kind="ExternalInput")
    t_off = nc.dram_tensor("off", [NPIX, 2 * KN], f32, kind="ExternalInput")
    t_grid = nc.dram_tensor("grid", [P, NT * 2 * KN], f32, kind="ExternalInput")
    t_w = nc.dram_tensor("w", [C, KN, F], f32, kind="ExternalInput")
    t_b = nc.dram_tensor("b", [F], f32, kind="ExternalInput")
    t_out = nc.dram_tensor("out", [NPIX, F], f32, kind="ExternalOutput")
    with tile.TileContext(nc) as tc:
        _body(tc, t_x, t_off, t_grid, t_w, t_b, t_out, debug=debug)
    nc.compile()
    return nc


def make_in_maps(x, offset, W, b):
    B = x.shape[0]
    grid_host = _grid_const()
    w_host = np.ascontiguousarray(np.asarray(W, np.float32).transpose(1, 0, 2))
    b_host = np.ascontiguousarray(np.asarray(b, np.float32))
    in_maps = []
    pad = np.zeros((1, C), np.float32)
    for i in range(B):
        xi = np.asarray(x[i], np.float32).reshape(NPIX, C)
        in_maps.append(
            {
                "x": np.ascontiguousarray(np.concatenate([xi, pad], axis=0)),
                "off": np.ascontiguousarray(
                    np.asarray(offset[i], np.float32).reshape(NPIX, 2 * KN)
                ),
                "grid": grid_host,
                "w": w_host,
                "b": b_host,
            }
        )
    return in_maps


_RESULTS_CACHE = {}


def kernel(x, offset, W, b, _trace=False):
    x = np.asarray(x)
    B = x.shape[0]
    assert x.shape == (B, H, W_IMG, C), x.shape
    nc = build_nc()
    in_maps = make_in_maps(x, offset, W, b)
    res = run_bass_kernel_spmd(nc, in_maps, core_ids=list(range(B)), trace=_trace)
    _RESULTS_CACHE["last"] = res
    out = np.stack(
        [res.results[i]["out"].reshape(H, W_IMG, F) for i in range(B)]
    ).astype(np.float32)
    return out
